# revision 2
# baseline (speedup 1.0000x reference)
"""Multi-head causal attention (QKV proj + attention + out proj) on 8 TRN2
NeuronCores — mixed fp16/fp8 flavor.

Sharding: 2-way data-parallel over batch x 4-way tensor-parallel over heads.
Core c handles batch c//4 and heads [4*(c%4), 4*(c%4)+4).

Precision scheme (validated vs reference on CPU, rel err ~1.4e-2 < 2e-2):
  - chunk 0 (q in [0,512), k-tiles 0-3) runs the fp16 pipeline: causal
    windows there are tiny, so quantization errors don't average out and
    fp8 anywhere in that path busts the error gate.
  - chunks 1-3 run fp8(e4m3) end to end: x, W, qT/kT, pt, V, o, Wo all
    fp8.  Big windows average the 4.4%-rms e4m3 noise down ~20x.
  - k-tiles 0-3 of kT/V are kept in BOTH dtypes (fp8 copies made from the
    fp16 evacs) since later chunks attend to them.

Speed changes vs the fp16 baseline (206.8us):
  - fp8 DoubleRow matmuls: projections contract 256 rows per MM (4 MMs
    instead of 8), PV processes TWO k-tiles per MM (stationary
    [128,2,65], moving [128,2,512]), out-proj is ONE MM per 128-col
    block.  ~2x PE throughput on those stages.
  - scores stay pair-packed 64-row matmuls (fp8 operands, same speed).
  - exp goes straight to fp8 via ACT (out dtype e4m3) or the DVE int8
    bit-trick exp(x) ~= bitcast_e4m3(int8(x*11.5416 + 56)) (|x| < 4.8).
  - diagonal k-tiles fuse exp+causal-mask in ONE DVE scalar_tensor_tensor:
    pt = int8((sc + 4.852) * (11.5416*mask)), restricted to the valid
    column range; the invalid rectangle is zeroed by GPSIMD memsets.
  - softmax denominators ride as PSUM row 64 via a ones column in V
    (stationary 65 cols); po pair evacuated in ONE [65,2,512] ACT copy.
"""

import numpy as np
import ml_dtypes
from collections import deque
from contextlib import ExitStack

import concourse.bass as bass
import concourse.mybir as mybir
import concourse.tile as tile
from concourse import bacc
from concourse.bass import ds
from concourse.bass_utils import run_bass_kernel_spmd

B, S_FULL, E, H = 2, 2048, 1024, 16
D = E // H          # 64
NCORES = 8
TP = 4              # tensor-parallel ways (over heads)
HL = H // TP        # 4 local heads per core
F = HL * D          # 256 local projection width
P = 128
QCH = 512           # q-chunk / matmul moving-dim size
DP8 = 80            # padded vo8 per-head stride (16B-aligned k-pair step)
DP16 = 66           # vo16 per-head stride
FP32 = mybir.dt.float32
F16 = mybir.dt.float16
F8 = mybir.dt.float8e4
I8 = mybir.dt.int8
I16 = mybir.dt.int16
AF = mybir.ActivationFunctionType
ALU = mybir.AluOpType
DR = mybir.MatmulPerfMode.DoubleRow
E4NP = ml_dtypes.float8_e4m3

RS2 = float(8.0 ** -0.5)        # 1/sqrt(8): scores scale split onto q and k
WSC = 32.0                      # host scale on fp8 weights
SCQK8 = 1.0 / (WSC / RS2)       # fp8 q/k evac: acc*(32)->(q)/sqrt8
TRK_M8, TRK_B8 = 11.541560, 56.0       # e4m3 exp bit-trick
TRK_BIAS8 = TRK_B8 / TRK_M8            # 4.85203
TRK_M16, TRK_B16 = 1477.3194, 15360.0  # fp16 exp bit-trick
ONORM = 16.0                    # o scaled by 16/denom for fp8 range


def build(S=S_FULL, causal=True):
    ET = E // P          # 8 contraction tiles for projections
    NQ = S // QCH        # 4 q chunks
    KT = S // P          # 16 k tiles
    KPQ = QCH // P       # 4 k tiles per q chunk

    nc = bacc.Bacc()

    def din(name, shape, dt):
        return nc.declare_dram_parameter(name, shape, dt, isOutput=False)

    # chunk 0 inputs fp16; chunks 1..NQ-1 fp8
    xq16 = din("xq16", [P, ET, QCH], F16)
    xk16 = din("xk16", [P, ET, QCH], F16)
    xv16 = din("xv16", [P, ET, QCH], F16)
    xq8 = din("xq8", [NQ - 1, P, ET, QCH], F8)
    xk8 = din("xk8", [NQ - 1, P, ET, QCH], F8)
    xv8 = din("xv8", [NQ - 1, P, ET, QCH], F8)
    wq16d = din("wq16", [P, ET, F], F16)   # pre-scaled by 1/sqrt8
    wk16d = din("wk16", [P, ET, F], F16)   # pre-scaled by 1/sqrt8
    wv16d = din("wv16", [P, ET, F], F16)
    wo16d = din("wo16", [P, F // P, E], F16)
    wq8d = din("wq8", [P, ET, F], F8)      # x32
    wk8d = din("wk8", [P, ET, F], F8)
    wv8d = din("wv8", [P, ET, F], F8)
    wo8d = din("wo8", [P, F // P, E], F8)
    bcat = din("bcat", [P, 2 + F], FP32)   # bq2 (x 1/sqrt8) | bvb
    m01 = din("m01", [P, KPQ, 2, QCH], F16)  # 0/1 causal masks per diag tile
    outT = nc.declare_dram_parameter("outT", [E, S], F16, isOutput=True)

    with ExitStack() as ctx:
        ctx.enter_context(
            nc.allow_low_precision(reason="fp8/fp16 matmuls are the design point")
        )
        tc = ctx.enter_context(tile.TileContext(nc))
        const = ctx.enter_context(tc.tile_pool(name="const", bufs=1))
        xp = ctx.enter_context(tc.tile_pool(name="xp", bufs=3))
        xp0 = ctx.enter_context(tc.tile_pool(name="xp0", bufs=1))
        ptp = ctx.enter_context(tc.tile_pool(name="ptp", bufs=4))
        dnp = ctx.enter_context(tc.tile_pool(name="dnp", bufs=3))
        otp = ctx.enter_context(tc.tile_pool(name="otp", bufs=3))
        # PSUM: sc 2x2 banks + acc 2 + po (pair tile) 2 = 8
        scp = ctx.enter_context(tc.tile_pool(name="scp", bufs=2, space="PSUM"))
        accp = ctx.enter_context(tc.tile_pool(name="accp", bufs=2, space="PSUM"))
        pop = ctx.enter_context(tc.tile_pool(name="pop", bufs=1, space="PSUM"))

        # ---- constants / persistent tensors ----
        # wq16 first: warm-up matmuls depend only on it.  Big early DMAs are
        # split across queues (one dma_start rides one queue at ~45GB/s).
        wq16 = const.tile([P, ET, F], F16)
        for h in range(4):
            nc.sync.dma_start(out=wq16[:, 2 * h:2 * h + 2, :],
                              in_=wq16d[:, 2 * h:2 * h + 2, :])
        wk16 = const.tile([P, ET, F], F16)
        for h in range(2):
            nc.sync.dma_start(out=wk16[:, 4 * h:4 * h + 4, :],
                              in_=wk16d[:, 4 * h:4 * h + 4, :])
        m01_sb = const.tile([P, KPQ, 2, QCH], F16)
        msk8_sb = const.tile([P, KPQ, 2, QCH], F16)  # 11.5416 * mask, device-built
        bcat_sb = const.tile([P, 2 + F], FP32)
        nc.sync.dma_start(out=bcat_sb, in_=bcat[:, :])
        bq_sb = bcat_sb[:, 0:2]
        bvb_sb = bcat_sb[:, 2:2 + F]
        wv16 = const.tile([P, ET, F], F16)
        wo16 = const.tile([P, F // P, E], F16)
        wq8 = const.tile([P, ET, F], F8)
        wk8 = const.tile([P, ET, F], F8)
        wv8 = const.tile([P, ET, F], F8)
        wo8 = const.tile([P, F // P, E], F8)

        # PE clock warm-up: enough to stay busy until the x16 DMA lands (an
        # idle MID window right before chunk-0 proj re-throttles the clock)
        for _ in range(22):
            wps = accp.tile([P, QCH], FP32, tag="acc")
            nc.tensor.matmul(
                wps,
                wq16[:, 0, 0:P],
                wq16[:, 0:2, :].rearrange("p a b -> p (a b)"),
                start=True, stop=True,
            )

        ones_f32 = const.tile([P, D], FP32)
        nc.vector.memset(ones_f32, 1.0)
        sixteen = const.tile([1, D], F16)
        nc.vector.memset(sixteen, ONORM)

        # q/k for chunk 0 (fp16) and all chunks (fp8)
        qT16 = const.tile([P, F // P, QCH], F16)
        kT16 = const.tile([P, F // P, QCH], F16)
        qT8 = const.tile([P, F // P, S], F8)
        kT8 = const.tile([P, F // P, S], F8)
        # V with ones column at index D (denominator trick)
        vo16 = const.tile([P, KPQ, HL, DP16], F16)
        vo8 = const.tile([P, KT, HL, DP8], F8)
        nc.scalar.activation(
            vo16[:, :, :, D:D + 1],
            ones_f32[:, 0:KPQ * HL].rearrange("p (a b c) -> p a b c", a=KPQ, b=HL, c=1),
            AF.Copy,
        )
        nc.scalar.activation(
            vo8[:, :, :, D:D + 1],
            ones_f32[:, 0:KT * HL].rearrange("p (a b c) -> p a b c", a=KT, b=HL, c=1),
            AF.Copy,
        )
        oT16 = const.tile([P, F // P, QCH], F16)
        oT8 = const.tile([P, F // P, S], F8)
        # unnormalized attention output + denominator row (row D), per head
        ou_all = const.tile([P, HL, S], F16)

        x_tiles = {}

        def emit_x_dma(j):
            if j == 0:
                for name, src in (("q", xq16), ("k", xk16), ("v", xv16)):
                    t = xp0.tile([P, ET, QCH], F16, tag=f"x{name}16")
                    for h in range(4):
                        nc.sync.dma_start(out=t[:, 2 * h:2 * h + 2, :],
                                          in_=src[:, 2 * h:2 * h + 2, :])
                    x_tiles[(name, 0)] = t
            else:
                for name, src in (("q", xq8), ("k", xk8), ("v", xv8)):
                    t = xp.tile([P, ET, QCH], F8, tag=f"x{name}8")
                    nc.sync.dma_start(out=t, in_=src[j - 1])
                    x_tiles[(name, j)] = t

        # ---- engine-balance helper for evac/exp work: ACT vs DVE ----
        ebal = {"act": 0.0, "dve": 0.0}

        def pick_engine(act_cost, dve_cost):
            if ebal["act"] + act_cost <= ebal["dve"] + dve_cost:
                ebal["act"] += act_cost
                return "act"
            ebal["dve"] += dve_cost
            return "dve"

        # ---- projection / out-projection unit generators (PE fillers) ----
        def proj_qk_unit(j, which, blk):
            xt = x_tiles[(which, j)]
            acc = accp.tile([P, QCH], FP32, tag="acc")
            if j == 0:
                w_sb = wq16 if which == "q" else wk16
                for et in range(ET):
                    nc.tensor.matmul(
                        acc,
                        w_sb[:, et, ds(blk * P, P)],
                        xt[:, et, :],
                        start=(et == 0),
                        stop=(et == ET - 1),
                    )
                dst16 = (qT16 if which == "q" else kT16)[:, blk, :]
                if which == "q":
                    nc.vector.tensor_scalar_add(dst16, acc, bq_sb[:, blk:blk + 1])
                else:
                    # k: no bias (cancels in softmax); also make the fp8 copy
                    nc.scalar.activation(dst16, acc, AF.Copy)
                    nc.scalar.activation(kT8[:, blk, ds(0, QCH)], acc, AF.Copy)
                return 2400
            w_sb = wq8 if which == "q" else wk8
            for ep in range(ET // 2):
                nc.tensor.matmul(
                    acc,
                    w_sb[:, 2 * ep:2 * ep + 2, ds(blk * P, P)],
                    xt[:, 2 * ep:2 * ep + 2, :],
                    start=(ep == 0),
                    stop=(ep == ET // 2 - 1),
                    perf_mode=DR,
                )
            dst = (qT8 if which == "q" else kT8)[:, blk, ds(j * QCH, QCH)]
            if which == "q":
                eng = pick_engine(620, 740)
                if eng == "act":
                    nc.scalar.activation(
                        dst, acc, AF.Identity,
                        bias=bq_sb[:, blk:blk + 1], scale=SCQK8,
                    )
                else:
                    nc.vector.tensor_scalar(
                        dst, acc, SCQK8, bq_sb[:, blk:blk + 1],
                        op0=ALU.mult, op1=ALU.add,
                    )
            else:
                eng = pick_engine(620, 740)
                if eng == "act":
                    nc.scalar.activation(dst, acc, AF.Copy, scale=SCQK8)
                else:
                    nc.vector.tensor_scalar(dst, acc, SCQK8, 0.0, op0=ALU.mult)
            return 1300

        def proj_v_unit(j, sl):
            xt = x_tiles[("v", j)]
            st = j * KPQ + sl
            acc = accp.tile([P, QCH], FP32, tag="acc")
            if j == 0:
                for et in range(ET):
                    nc.tensor.matmul(
                        acc[:, 0:F],
                        xt[:, et, ds(sl * P, P)],
                        wv16[:, et, :],
                        start=(et == 0),
                        stop=(et == ET - 1),
                    )
                nc.vector.tensor_add(
                    vo16[:, st, :, 0:D],
                    acc[:, 0:F].rearrange("p (h d) -> p h d", h=HL),
                    bvb_sb.rearrange("p (h d) -> p h d", h=HL),
                )
                # fp8 copy for later chunks
                nc.scalar.activation(
                    vo8[:, st, :, 0:D], vo16[:, st, :, 0:D], AF.Copy
                )
                return 1300
            for ep in range(ET // 2):
                nc.tensor.matmul(
                    acc[:, 0:F],
                    xt[:, 2 * ep:2 * ep + 2, ds(sl * P, P)],
                    wv8[:, 2 * ep:2 * ep + 2, :],
                    start=(ep == 0),
                    stop=(ep == ET // 2 - 1),
                    perf_mode=DR,
                )
            nc.vector.scalar_tensor_tensor(
                vo8[:, st, :, 0:D],
                acc[:, 0:F].rearrange("p (h d) -> p h d", h=HL),
                1.0 / WSC,
                bvb_sb.rearrange("p (h d) -> p h d", h=HL),
                op0=ALU.mult, op1=ALU.add,
            )
            return 700

        def outproj_unit(j, eb):
            acc = accp.tile([P, QCH], FP32, tag="acc")
            if j == 0:
                for fb in range(F // P):
                    nc.tensor.matmul(
                        acc,
                        wo16[:, fb, ds(eb * P, P)],
                        oT16[:, fb, :],
                        start=(fb == 0),
                        stop=(fb == F // P - 1),
                    )
                oscale = 1.0 / ONORM
            else:
                nc.tensor.matmul(
                    acc,
                    wo8[:, 0:2, ds(eb * P, P)],
                    oT8[:, 0:2, ds(j * QCH, QCH)],
                    start=True, stop=True,
                    perf_mode=DR,
                )
                oscale = 1.0 / (ONORM * WSC)
            ot = otp.tile([P, QCH], F16, tag="ot")
            eng = pick_engine(620, 660)
            if eng == "act":
                nc.scalar.activation(ot, acc, AF.Copy, scale=oscale)
            else:
                nc.vector.tensor_scalar(ot, acc, oscale, 0.0, op0=ALU.mult)
            nc.sync.dma_start(out=outT[ds(eb * P, P), ds(j * QCH, QCH)], in_=ot)
            return 400 if j else 700

        fillers = deque()   # projection units: MUST drain by the next chunk
        out_q = deque()     # out-projection units: consumed lazily, late

        def emit_dummy_mm():
            wps = accp.tile([P, QCH], FP32, tag="acc")
            nc.tensor.matmul(
                wps,
                wq16[:, 0, 0:P],
                wq16[:, 0:2, :].rearrange("p a b -> p (a b)"),
                start=True, stop=True,
            )
            return 320

        def emit_tiny_mm():
            # keep-warm micro-matmul: ~80ns of PE activity so the HAM MID
            # window never sees the PE fully idle (re-throttle costs far more)
            wps = accp.tile([P, QCH], FP32, tag="acc")
            nc.tensor.matmul(
                wps[:, 0:D],
                wq16[:, 0, 0:P],
                wq16[:, 0, 0:D],
                start=True, stop=True,
            )
            return 80

        use_outq = [False]   # out_q held in reserve for the exp-bound tail

        def do_filler(budget, pad=0):
            while budget > 0 and fillers:
                budget -= fillers.popleft()()
            while budget > 0 and out_q and use_outq[0]:
                budget -= out_q.popleft()()
            pad = min(budget, pad)
            while pad > 0:
                pad -= emit_dummy_mm()

        def drain_fillers():
            while fillers:
                fillers.popleft()()

        def drain_all():
            drain_fillers()
            while out_q:
                out_q.popleft()()

        def push_proj(j):
            for blk in range(F // P):
                fillers.append(lambda j=j, b=blk: proj_qk_unit(j, "q", b))
                fillers.append(lambda j=j, b=blk: proj_qk_unit(j, "k", b))
            for sl in range(KPQ):
                fillers.append(lambda j=j, s=sl: proj_v_unit(j, s))

        def push_outproj(j):
            for eb in range(E // P):
                out_q.append(lambda j=j, e=eb: outproj_unit(j, e))

        # ---- normalization ----
        def emit_evac(j, pr, po_t):
            # one [65, 2, 512] copy for the pair (po banks are adjacent)
            nc.scalar.activation(
                ou_all[0:D + 1, 2 * pr:2 * pr + 2, ds(j * QCH, QCH)],
                po_t[0:D + 1, :, :],
                AF.Copy,
            )

        def norm_phase1(j, pr):
            PPH = QCH // 16  # 32 partitions per head's denominator row
            dn = dnp.tile([2 * PPH, 16], F16, tag="dn")
            for i in range(2):
                nc.sync.dma_start(
                    out=dn[i * PPH:(i + 1) * PPH, :],
                    in_=ou_all[D:D + 1, 2 * pr + i, ds(j * QCH, QCH)],
                )
            rc = dnp.tile([2 * PPH, 16], F16, tag="rc")
            nc.vector.reciprocal(rc, dn)
            rcr = dnp.tile([1, 2, QCH], F16, tag="rcr")
            for i in range(2):
                nc.sync.dma_start(
                    out=rcr[:, i, :], in_=rc[i * PPH:(i + 1) * PPH, :]
                )
            return rcr

        def norm_phase2(j, pr, rcr):
            for i in range(2):
                h = 2 * pr + i
                doff = i * D
                bc = accp.tile([P, QCH], FP32, tag="acc")
                nc.tensor.matmul(
                    bc[0:D, :], sixteen[0:1, :], rcr[:, i, :],
                    start=True, stop=True,
                )
                if j == 0:
                    dst = oT16[doff:doff + D, pr, :]
                else:
                    dst = oT8[doff:doff + D, pr, ds(j * QCH, QCH)]
                nc.vector.tensor_mul(
                    dst, ou_all[0:D, h, ds(j * QCH, QCH)], bc[0:D, :]
                )

        def emit_norm_pair(j, pr):
            norm_phase2(j, pr, norm_phase1(j, pr))

        # ---- main emission loop ----
        # DMA order = arrival-deadline order: x16+wv16 feed chunk-0 proj,
        # m01 feeds chunk-0 attention, x8(1)+w8 feed chunk-1 proj fillers
        # (which run during chunk-0 attention), wo16 feeds outproj(0).
        emit_x_dma(0)
        nc.sync.dma_start(out=wv16, in_=wv16d[:, :, :])
        nc.sync.dma_start(out=m01_sb, in_=m01[:, :, :, :])
        emit_x_dma(1)
        nc.sync.dma_start(out=wq8, in_=wq8d[:, :, :])
        nc.sync.dma_start(out=wk8, in_=wk8d[:, :, :])
        nc.sync.dma_start(out=wv8, in_=wv8d[:, :, :])
        emit_x_dma(2)
        emit_x_dma(3)
        nc.sync.dma_start(out=wo16, in_=wo16d[:, :, :])
        nc.sync.dma_start(out=wo8, in_=wo8d[:, :, :])
        push_proj(0)
        drain_fillers()          # projections for chunk 0 up front
        # device-built fp8 exp+mask constant: 11.5416 * m01
        nc.vector.tensor_scalar(
            msk8_sb.rearrange("p a b c -> p (a b c)"),
            m01_sb.rearrange("p a b c -> p (a b c)"),
            TRK_M8, 0.0, op0=ALU.mult,
        )

        pending = None
        for j in range(NQ):
            if j + 1 < NQ:
                push_proj(j + 1)
            nkt = KPQ * (j + 1) if causal else KT
            for pr in range(HL // 2):
                hA, hB = 2 * pr, 2 * pr + 1
                po = pop.tile([P, 2, QCH], FP32, tag="po")
                pt8_t = None
                pv_pending = None   # PV emission delayed one k-tile (pair) so
                # the exp feeding it has a full extra slot of latency slack
                for kt in range(nkt):
                    sc = scp.tile([P, 2, QCH], FP32, tag="sc")
                    if j == 0:
                        kTs, qTs = kT16[:, :, :], qT16[:, :, :]
                        qoff = 0
                    else:
                        kTs, qTs = kT8[:, :, :], qT8[:, :, :]
                        qoff = j * QCH
                    nc.tensor.matmul(
                        sc[:, 0, :],
                        kTs[0:D, pr, ds(kt * P, P)],
                        qTs[0:D, pr, ds(qoff, QCH)],
                        start=True, stop=True,
                        tile_position=(0, 0),
                    )
                    nc.tensor.matmul(
                        sc[:, 1, :],
                        kTs[D:P, pr, ds(kt * P, P)],
                        qTs[D:P, pr, ds(qoff, QCH)],
                        start=True, stop=True,
                        tile_position=(64, 0),
                    )
                    diag = causal and kt >= KPQ * j
                    t = kt - KPQ * j
                    if j == 0:
                        # fp16 path: ACT exp on the valid columns + triangle
                        # mask-mul (DVE / GPSIMD alternating)
                        w0 = t * P
                        pt16_t = ptp.tile([P, 2, QCH], F16, tag="pt16")
                        if w0 > 0:
                            nc.gpsimd.memset(pt16_t[:, :, 0:w0], 0.0)
                        nc.scalar.activation(
                            pt16_t[:, :, w0:QCH], sc[:, :, w0:QCH], AF.Exp
                        )
                        ebal["act"] += 240 + (QCH - w0) * 2 * 0.85
                        tri_in = pt16_t[:, :, w0:w0 + P]
                        trimask = m01_sb[:, t, :, w0:w0 + P]
                        if t % 2 == 0:
                            nc.gpsimd.tensor_mul(tri_in, tri_in, trimask)
                        else:
                            nc.vector.tensor_mul(tri_in, tri_in, trimask)
                            ebal["dve"] += 380
                        do_filler(700)
                        if pv_pending is not None:
                            pv_pending()
                        def pv16(kt=kt, pt16_t=pt16_t, po=po):
                            for i, h in ((0, hA), (1, hB)):
                                nc.tensor.matmul(
                                    po[0:D + 1, i, :],
                                    vo16[:, kt, h, 0:D + 1],
                                    pt16_t[:, i, :],
                                    start=(kt == 0),
                                    stop=(kt == nkt - 1),
                                )
                        pv_pending = pv16
                        continue
                    # fp8 path: pt pairs [P, 2kt, 2hd, QCH]
                    if kt % 2 == 0:
                        pt8_t = ptp.tile([P, 2, 2, QCH], F8, tag="pt8")
                    ptslice = pt8_t[:, kt % 2, :, :]
                    if diag:
                        w0 = t * P
                        if w0 > 0:
                            nc.gpsimd.memset(ptslice[:, :, 0:w0], 0.0)
                        if t % 2 == 0:
                            # ACT exp + GPSIMD triangle mask
                            nc.scalar.activation(
                                ptslice[:, :, w0:QCH], sc[:, :, w0:QCH], AF.Exp
                            )
                            ebal["act"] += 240 + (QCH - w0) * 2 * 0.85
                            nc.gpsimd.tensor_mul(
                                ptslice[:, :, w0:w0 + P],
                                ptslice[:, :, w0:w0 + P],
                                m01_sb[:, t, :, w0:w0 + P],
                            )
                        else:
                            # fused exp+mask on DVE
                            nc.vector.scalar_tensor_tensor(
                                ptslice[:, :, w0:QCH].bitcast(I8),
                                sc[:, :, w0:QCH],
                                TRK_BIAS8,
                                msk8_sb[:, t, :, w0:QCH],
                                op0=ALU.add, op1=ALU.mult,
                            )
                            ebal["dve"] += 240 + (QCH - w0) * 2 * 1.05
                    else:
                        eng = pick_engine(1090, 1215)
                        if eng == "act":
                            nc.scalar.activation(ptslice, sc, AF.Exp)
                        else:
                            nc.vector.tensor_scalar(
                                ptslice.bitcast(I8), sc, TRK_M8, TRK_B8,
                                op0=ALU.mult, op1=ALU.add,
                            )
                    do_filler(700)
                    if kt % 2 == 1:
                        if pv_pending is not None:
                            pv_pending()
                        def pv8(kt=kt, pt8_t=pt8_t, po=po):
                            for i, h in ((0, hA), (1, hB)):
                                nc.tensor.matmul(
                                    po[0:D + 1, i, :],
                                    vo8[:, kt - 1:kt + 1, h, 0:D + 1],
                                    pt8_t[:, :, i, :],
                                    start=(kt == 1),
                                    stop=(kt == nkt - 1),
                                    perf_mode=DR,
                                )
                        pv_pending = pv8
                if pv_pending is not None:
                    pv_pending()
                emit_evac(j, pr, po)
                ebal["act"] += 1060
                if pending is not None:
                    pj, ppr = pending
                    emit_norm_pair(pj, ppr)
                    ebal["dve"] += 1500
                    if ppr == 1:
                        push_outproj(pj)
                pending = (j, pr)
            drain_fillers()
            if j == NQ - 2:
                # release the banked out-projections into the (exp-bound)
                # final chunk's PE stalls
                use_outq[0] = True
        # final norm: emit the denominator DMA round trip, pad the PE while
        # it is in flight (in-order engine: pads must be emitted BEFORE the
        # bc matmul), then the PE-side of the norm and the out-projection.
        rcr = norm_phase1(*pending)
        do_filler(9000, pad=9000)
        norm_phase2(*pending, rcr)
        push_outproj(NQ - 1)
        drain_all()

    nc.compile()
    return nc


def make_masks(S=S_FULL):
    KPQ = QCH // P
    m = np.zeros((P, KPQ, QCH), np.float32)
    for t in range(KPQ):
        kk = np.arange(P)[:, None]
        qq = np.arange(QCH)[None, :]
        m[:, t, :] = (qq >= kk + P * t).astype(np.float32)
    return m


def make_in_maps(query, key, value, Wq, bq, Wk, bk, Wv, bv, Wo, bo, S=S_FULL):
    ET = E // P
    NQ = S // QCH
    q = np.asarray(query, np.float32)
    k = np.asarray(key, np.float32)
    v = np.asarray(value, np.float32)
    Wq = np.asarray(Wq, np.float32)
    Wk = np.asarray(Wk, np.float32)
    Wv = np.asarray(Wv, np.float32)
    Wo = np.asarray(Wo, np.float32)
    bq = np.asarray(bq, np.float32)
    bv = np.asarray(bv, np.float32)

    def xswiz(xT):
        # [E, S] -> [NQ, P, ET, QCH]
        return np.ascontiguousarray(
            xT.reshape(ET, P, NQ, QCH).transpose(2, 1, 0, 3)
        )

    def wswiz(wT):
        # [E, F] -> [P, ET, F]
        return np.ascontiguousarray(wT.reshape(ET, P, F).transpose(1, 0, 2))

    masks = make_masks(S)
    m01 = np.ascontiguousarray(
        np.broadcast_to(masks[:, :, None, :], (P, QCH // P, 2, QCH))
    ).astype(np.float16)
    in_maps = []
    for c in range(NCORES):
        b, tp = divmod(c, TP)
        rows = slice(tp * F, (tp + 1) * F)
        bq2 = (bq[rows] * RS2).reshape(F // P, P).T        # [P, 2]
        bvb = np.broadcast_to(bv[rows], (P, F))            # [P, F]
        bcat = np.concatenate([bq2, bvb], axis=1).astype(np.float32)
        woT = Wo[:, rows].T                                # [F, E]
        wo3 = woT.reshape(F // P, P, E).transpose(1, 0, 2)
        xq = xswiz(q[b].T)
        xk = xswiz(k[b].T)
        xv = xswiz(v[b].T)
        in_maps.append({
            "xq16": xq[0].astype(np.float16),
            "xk16": xk[0].astype(np.float16),
            "xv16": xv[0].astype(np.float16),
            "xq8": xq[1:].astype(E4NP),
            "xk8": xk[1:].astype(E4NP),
            "xv8": xv[1:].astype(E4NP),
            "wq16": wswiz(Wq[rows].T * RS2).astype(np.float16),
            "wk16": wswiz(Wk[rows].T * RS2).astype(np.float16),
            "wv16": wswiz(Wv[rows].T).astype(np.float16),
            "wo16": np.ascontiguousarray(wo3).astype(np.float16),
            "wq8": wswiz(Wq[rows].T * WSC).astype(E4NP),
            "wk8": wswiz(Wk[rows].T * WSC).astype(E4NP),
            "wv8": wswiz(Wv[rows].T * WSC).astype(E4NP),
            "wo8": np.ascontiguousarray(wo3 * WSC).astype(E4NP),
            "bcat": np.ascontiguousarray(bcat),
            "m01": m01,
        })
    return in_maps


_CACHE = {}


def _get_nc(causal):
    if causal not in _CACHE:
        _CACHE[causal] = build(S_FULL, causal)
    return _CACHE[causal]


def kernel(query, key, value, Wq, bq, Wk, bk, Wv, bv, Wo, bo, is_causal):
    causal = bool(int(np.asarray(is_causal)))
    nc = _get_nc(causal)
    in_maps = make_in_maps(query, key, value, Wq, bq, Wk, bk, Wv, bv, Wo, bo)
    res = run_bass_kernel_spmd(nc, in_maps, core_ids=list(range(NCORES)))
    out = np.zeros((B, S_FULL, E), np.float32)
    for c in range(NCORES):
        b, tp = divmod(c, TP)
        out[b] += res.results[c]["outT"].T.astype(np.float32)
    out += np.asarray(bo, np.float32)
    return out
